# revision 82
# baseline (speedup 1.0000x reference)
"""Trainium2 Bass kernel for nn_AlwaysLayer (soft-product rule layer).

Math (per reference):
  x: [B=4096, D=4096] f32, w_start/w_end: [2048] f32
  K=64 groups, J=BIN1=8, T=BIN_T=8, N=4
  W[k,j,n,t] = sigmoid((t - ws[k,j,n]) * (we[k,j,n] - t))
  X[b,k,j,n,t] = 1 - (1 - x[b,k,t,j]) * W[k,j,n,t]  = a + w*x  (a=1-W, w=W)
  y[b,k,j,n] = -1/(-1 + sum_t log X) = 1/(1 - log(prod_t X))
  out: [B, 2048] (k,j,n order)

Strategy: data-parallel over batch across 8 cores (512 rows each), no
collectives. Per core: PE-transpose x into [(k,j) partitions, (t,b) free]
layout so W/A become per-partition scalar columns; fused mult-add via
tensor_scalar (DVE) / activation (ACT); product over t as a bf16
tensor_tensor tree in the free axis; one Ln per output; PE-transpose back
to [b, (k,j,n)] rows; reciprocal there; store.

The walrus codegen used by the axon terminal allows AT MOST ONE embedded
sync wait per TPB instruction (PE matmul, ACT/DVE ops, gpsimd ucode ops,
DMA). Tile-pool slot recycling always emits sem waits, while rewriting
the SAME tile from the same engine does not. So:
  - every multi-use buffer is a manually double-buffered singleton tile;
  - all PE-matmul inputs are produced on ACT (waits merge onto the
    Activation semaphore);
  - DMA-landed constants are laundered through ACT copies;
  - tiny ACT "observer" copies pre-absorb cross-engine WAR waits so the
    following real instruction needs only its RAW wait.
"""

import numpy as np

import concourse.bass as bass
import concourse.mybir as mybir
import concourse.tile as tile
from concourse import masks
from concourse.bass_utils import run_bass_kernel_spmd

F32 = mybir.dt.float32
BF16 = mybir.dt.bfloat16
ALU = mybir.AluOpType
AF = mybir.ActivationFunctionType

N_CORES = 8
B_FULL = 4096
D = 4096
K = 64          # groups
J = 8           # BIN1
T = 8           # BIN_T
N = 4           # rules per (group, bin1)
B = B_FULL // N_CORES       # 512 rows per core
KT = 4          # k-tiles (16 k each -> 128 partitions of (k16, j))
K_PER_KT = K // KT          # 16
BT = B // 128   # 4 b-tiles per core
F_OUT = K * J * N           # 2048

# dtype of the term tensor fed to the product tree (bf16 = 2x DVE TT mode)
TERM_DT = BF16
# dtype of transposed x (f32 keeps the FMA exact; bf16 enables 4x TS mode)
XT_DT = BF16
# how many of the 8 per-(ktile,n) FMA t-slices run on ACT instead of DVE
# (must be even; they are the trailing t's and pair among themselves)
ACT_T = 2

_graph_cache = {}


def _build_graph(repeat=1):
    if repeat in _graph_cache:
        return _graph_cache[repeat]

    nc = bass.Bass()
    x_d = nc.declare_dram_parameter("x", [B, D], F32, isOutput=False)
    wv_d = nc.declare_dram_parameter("wv", [K * J, N * T], F32, isOutput=False)
    av_d = nc.declare_dram_parameter("av", [K * J, N * T], F32, isOutput=False)
    y_d = nc.declare_dram_parameter("out", [B, F_OUT], F32, isOutput=True)

    with tile.TileContext(nc) as tc:
        with (
            tc.tile_pool(name="const", bufs=1) as cpool,
            tc.tile_pool(name="sing", bufs=1) as spool,
            tc.tile_pool(name="psing", bufs=1, space="PSUM") as ppool,
        ):
            # ---- constants ----
            ident0 = cpool.tile([128, 128], F32, tag="ident0")
            masks.make_identity(nc, ident0[:])
            gscr = cpool.tile([1, 1], F32, tag="gscr")
            pool_term = nc.gpsimd.memset(gscr[:], 0.0)
            ident = cpool.tile([128, 128], F32, tag="ident")
            nc.scalar.copy(ident[:], ident0[:])

            wv0 = cpool.tile([128, KT, N * T], F32, tag="wv0")
            av0 = cpool.tile([128, KT, N * T], F32, tag="av0")
            wv_dma = nc.sync.dma_start(
                wv0[:], wv_d[:].rearrange("(kt p) c -> p kt c", kt=KT, p=128))
            av_dma = nc.sync.dma_start(
                av0[:], av_d[:].rearrange("(kt p) c -> p kt c", kt=KT, p=128))
            # launder DMA-landed params through ACT so consumers wait on ACT
            wv_sb = cpool.tile([128, KT, N * T], F32, tag="wv")
            av_sb = cpool.tile([128, KT, N * T], F32, tag="av")
            nc.scalar.copy(wv_sb[:], wv0[:])
            nc.scalar.copy(av_sb[:], av0[:])

            # observer scratch: each observer writes its own column, source
            # is the ACT-produced identity (always observed)
            n_obs = max(64, repeat * KT * BT * 8)
            ascr = cpool.tile([1, n_obs], F32, tag="ascr")
            dscr = cpool.tile([1, n_obs], F32, tag="dscr")
            pscr = cpool.tile([1, n_obs], F32, tag="pscr")
            obs_col = [0]
            dobs_col = [0]
            pobs_col = [0]

            def act_observe(dep_ins):
                """Tiny ACT op whose only wait is `dep_ins`'s semaphore;
                pre-observes that tick on the ACT engine."""
                o = nc.scalar.copy(
                    ascr[0:1, obs_col[0]:obs_col[0] + 1], ident[0:1, 0:1])
                obs_col[0] += 1
                tile.add_dep_helper(o.ins, dep_ins.ins, reason="act observe")
                return o

            def pool_observe(dep_ins):
                """Same, for the POOL (gpsimd) engine."""
                o = nc.gpsimd.memset(
                    pscr[0:1, pobs_col[0]:pobs_col[0] + 1], 0.0)
                pobs_col[0] += 1
                tile.add_dep_helper(o.ins, dep_ins.ins, reason="pl observe")
                return o

            def dve_observe(dep_ins):
                """Same, for the DVE engine."""
                o = nc.vector.tensor_copy(
                    dscr[0:1, dobs_col[0]:dobs_col[0] + 1], ident[0:1, 0:1])
                dobs_col[0] += 1
                tile.add_dep_helper(o.ins, dep_ins.ins, reason="dve observe")
                return o

            # ---- singleton working tiles ----
            xb_s, xb2_s, xt_s, term_s, t1_s, t2_s = [], [], [], [], [], []
            pall_s, s_s, ybu_s, ybt_s, psin_s, psout_s = [], [], [], [], [], []
            for i in range(2):
                xb_i = spool.tile([128, 2, BT, 1024], F32, tag=f"xb{i}")
                xb_s.append(xb_i)
                xb2_i = spool.tile([128, T, 128], F32, tag=f"xb2{i}")
                xb2_s.append(xb2_i)
                xt_i = spool.tile([128, T, B], XT_DT, tag=f"xt{i}")
                xt_s.append(xt_i)
                term_i = spool.tile([128, T, B], TERM_DT, tag=f"term{i}")
                term_s.append(term_i)
                t1_i = spool.tile([128, 4, B], TERM_DT, tag=f"t1{i}")
                t1_s.append(t1_i)
                t2_i = spool.tile([128, 2, B], TERM_DT, tag=f"t2{i}")
                t2_s.append(t2_i)
                pall_i = spool.tile([128, N, B], TERM_DT, tag=f"pall{i}")
                pall_s.append(pall_i)
                s_i = spool.tile([128, N, B], F32, tag=f"s{i}")
                s_s.append(s_i)
                ybu_i = spool.tile([128, 512], F32, tag=f"ybu{i}")
                ybu_s.append(ybu_i)
                ybt_i = spool.tile([128, 512], F32, tag=f"ybt{i}")
                ybt_s.append(ybt_i)
                psin_i = ppool.tile([128, T, 128], F32, tag=f"psin{i}")
                psin_s.append(psin_i)
                psout_i = ppool.tile([128, N, 128], F32, tag=f"psout{i}")
                psout_s.append(psout_i)
            ybr_s = []
            for i in range(2):
                ybr_i = spool.tile([128, BT, 512], F32, tag=f"ybr{i}")
                ybr_s.append(ybr_i)
            pescr = ppool.tile([1, 1], F32, tag="pescr")

            # ---- load all of x upfront (4 disjoint-region DMAs) ----
            load_dmas = []
            for kt_ in range(KT):
                ld = nc.gpsimd.dma_start(
                    xb_s[kt_ // 2][:, kt_ % 2],
                    x_d[:, kt_ * 1024:(kt_ + 1) * 1024].rearrange(
                        "(bt b) f -> b bt f", bt=BT, b=128),
                )
                load_dmas.append(ld)
            # pre-observe each load DMA's completion on the POOL proc so
            # first-touch reorder copies keep a single wait
            for ld in load_dmas:
                pool_observe(ld)
            # prime PE's view of the ACT-built identity once
            d0 = nc.tensor.matmul(
                pescr[0:1, 0:1], ident[:, 0:1], ident[:, 0:1],
                start=True, stop=True)

            NKG = KT * repeat
            last_fma = [None] * NKG    # last DVE FMA per ktile-pass
            last_afma = [None] * NKG   # last ACT FMA per ktile-pass
            last_recip = [None] * (NKG * BT)
            last_store = [None] * NKG
            last_xtcopy = [None] * (NKG * BT)
            last_l1b = [None, None]
            last_l3p = [None, None]
            last_ln = [None, None]
            last_scat = [None] * (NKG * BT)
            last_pts = [None] * (NKG * BT)
            last_inT = [None] * (NKG * BT)
            last_outT = [None] * NKG
            last_pe = [None]

            for kg in range(NKG):
                rep, kt = divmod(kg, KT)
                # ---- in-transpose: xT[p=(k16,j), (t, b)] ----
                xt_t = xt_s[kg % 2]
                xb_t = xb_s[kt // 2]
                if kg >= 2 and last_fma[kg - 2] is not None:
                    # pre-absorb the xt WAR (DVE + ACT FMA readers of kg-2)
                    act_observe(last_fma[kg - 2])
                    act_observe(last_afma[kg - 2])
                for bt in range(BT):
                    g = kg * BT + bt
                    # reorder (k,t,j)->(t,k,j) so each PE-transpose slice is
                    # contiguous (stationary matmul operand: one free dim);
                    # on the otherwise-idle gpsimd engine. Its PE WAR is
                    # pre-absorbed on the POOL proc.
                    xb2_t = xb2_s[g % 2]
                    if g >= 2 and last_inT[g - 2] is not None:
                        pool_observe(last_inT[g - 2])
                    rc = nc.gpsimd.tensor_copy(
                        xb2_t[:],
                        xb_t[:, kt % 2, bt, :].rearrange(
                            "b (k t j) -> b t k j", k=K_PER_KT, t=T, j=J))
                    # PE-side: a dummy matmul into the singleton psum scr
                    # absorbs the Pool RAW so the transposes keep only
                    # their ACT (psin WAR) wait
                    d = nc.tensor.matmul(
                        pescr[0:1, 0:1], xb2_t[:, 0, 0:1], ident[:, 0:1],
                        start=True, stop=True)
                    ps = psin_s[g % 2]
                    for t in range(T):
                        mm_in = nc.tensor.transpose(
                            ps[:, t, :], xb2_t[:, t, :], ident[:])
                        if t == 0:
                            tile.add_dep_helper(mm_in.ins, d.ins, sync=False,
                                                reason="dummy first")
                    last_inT[g] = mm_in
                    if g >= 2 and last_xtcopy[g - 2] is not None:
                        # pre-absorb the psin WAR (ACT-self, async compl.)
                        act_observe(last_xtcopy[g - 2])
                    xc = nc.scalar.copy(
                        xt_t[:, :, bt * 128:(bt + 1) * 128], ps[:])
                    last_xtcopy[g] = xc

                # ---- FMA + product tree per n ----
                pall = pall_s[kg % 2]
                for n in range(N):
                    term = term_s[n % 2]
                    if last_l1b[n % 2] is not None:
                        # pre-absorb the term WAR (DVE tree reader) before
                        # the ACT FMA slices rewrite it
                        act_observe(last_l1b[n % 2])
                    for t in range(T):
                        w_ap = wv_sb[:, kt, n * T + t:n * T + t + 1]
                        a_ap = av_sb[:, kt, n * T + t:n * T + t + 1]
                        if t < T - ACT_T:
                            f = nc.vector.tensor_scalar(
                                term[:, t, :], xt_t[:, t, :],
                                w_ap, a_ap, ALU.mult, ALU.add)
                            last_fma[kg] = f
                        else:
                            af = nc.scalar.activation(
                                term[:, t, :], xt_t[:, t, :],
                                AF.Identity, bias=a_ap, scale=w_ap)
                            last_afma[kg] = af
                    t1 = t1_s[n % 2]
                    nc.vector.tensor_tensor(
                        t1[:, 0:2, :], term[:, 0:2, :], term[:, 2:4, :],
                        ALU.mult)
                    # pre-absorb the ACT-FMA RAW on DVE so the mixed-input
                    # tree op keeps a single wait
                    dve_observe(last_afma[kg])
                    l1b = nc.vector.tensor_tensor(
                        t1[:, 2:4, :], term[:, 4:6, :], term[:, 6:8, :],
                        ALU.mult)
                    last_l1b[n % 2] = l1b
                    t2 = t2_s[n % 2]
                    if last_l3p[n % 2] is not None:
                        # pre-absorb the t2 WAR (POOL lvl3 reader) on DVE
                        dve_observe(last_l3p[n % 2])
                    l2 = nc.vector.tensor_tensor(
                        t2[:], t1[:, 0:2, :], t1[:, 2:4, :], ALU.mult)
                    # tree level 3 on the gpsimd engine: pre-absorb its RAW
                    # (DVE lvl2) and the pall WAR (ACT Ln of kg-2) on POOL
                    pool_observe(l2)
                    if last_ln[kg % 2] is not None:
                        pool_observe(last_ln[kg % 2])
                    l3 = nc.gpsimd.tensor_tensor(
                        pall[:, n, :], t2[:, 0, :], t2[:, 1, :], ALU.mult)
                    last_l3p[n % 2] = l3

                # ---- S = ln P on ACT (one batched op); the out-transpose
                # moves S itself and the 1-S affine runs post-transpose on
                # the otherwise-idle gpsimd engine ----
                s_t = s_s[kg % 2]
                if kg >= 2 and last_outT[kg - 2] is not None:
                    # pre-absorb the s WAR (PE out-transposes of kg-2)
                    act_observe(last_outT[kg - 2])
                ln = nc.scalar.activation(s_t[:], pall[:], AF.Ln)
                last_ln[kg % 2] = ln
                u_t = s_t

                # ---- out-transpose + reciprocal + store ----
                if kg >= 2 and last_store[kg - 2] is not None:
                    # pre-absorb the ybr WAR (store DMA of kg-2) on DVE
                    dve_observe(last_store[kg - 2])
                for bt in range(BT):
                    g = kg * BT + bt
                    if g >= 2 and last_pts[g - 2] is not None:
                        # pre-absorb the ybu WAR (pool affine reader of g-2)
                        act_observe(last_pts[g - 2])
                    pso = psout_s[g % 2]
                    for n in range(N):
                        mm = nc.tensor.transpose(
                            pso[:, n, :],
                            u_t[:, n, bt * 128:(bt + 1) * 128], ident[:])
                        last_pe[0] = mm
                        last_outT[kg] = mm
                    # psum free layout (n, p); rows need (p, n) interleave
                    ybu_t = ybu_s[g % 2]
                    if g >= 2 and last_scat[g - 2] is not None:
                        # pre-absorb the psout WAR (ACT-self, async compl.)
                        act_observe(last_scat[g - 2])
                    sc = nc.scalar.copy(
                        ybu_t[:].rearrange("b (p n) -> b n p", n=N, p=128),
                        pso[:])
                    last_scat[g] = sc
                    # 1 - S on gpsimd (RAW on scatter and WAR on the DVE
                    # recip both pre-absorbed on POOL), reciprocal on DVE
                    pool_observe(sc)
                    if g >= 2 and last_recip[g - 2] is not None:
                        pool_observe(last_recip[g - 2])
                    ybt_t = ybt_s[g % 2]
                    pt = nc.gpsimd.tensor_scalar(
                        ybt_t[:], ybu_t[:], -1.0, 1.0, ALU.mult, ALU.add)
                    last_pts[g] = pt
                    dve_observe(pt)
                    r = nc.vector.reciprocal(
                        ybr_s[kg % 2][:, bt, :], ybt_t[:])
                    last_recip[g] = r
                # one store per ktile: 4 loads + 4 stores = 8 SWDGE DMAs,
                # one per queue, no queue-ring predecessor waits. Benchmark
                # repeats (rep >= 1) recompute without storing.
                if rep == 0:
                    st = nc.gpsimd.dma_start(
                        y_d[:, kt * 512:(kt + 1) * 512].rearrange(
                            "(bt b) c -> b bt c", bt=BT, b=128),
                        ybr_s[kg % 2][:])
                    last_store[kg] = st

            # ---- pre-drain absorption: the Tile tail drain waits on the
            # whole global clock (14 sems), far beyond the one-wait struct
            # limit. SP nops each absorb one semaphore onto the SP observed
            # clock so the drain itself needs no waits. ----
            terminals = ([wv_dma, av_dma, pool_term] + load_dmas
                         + [s for s in last_store if s is not None]
                         + [last_pe[0], last_scat[NKG * BT - 1],
                            last_recip[NKG * BT - 1]])
            for dep in terminals:
                sp_nop = nc.sync.nop(nofuse=True)
                tile.add_dep_helper(sp_nop.ins, dep.ins,
                                    reason="pre-drain sem absorb")
    _graph_cache[repeat] = nc
    return nc


def _host_prep(w_start, w_end):
    ws = w_start.astype(np.float64).reshape(K, J, N)
    we = w_end.astype(np.float64).reshape(K, J, N)
    t = np.arange(T, dtype=np.float64)
    z = (t[None, None, None, :] - ws[..., None]) * (we[..., None] - t[None, None, None, :])
    W = 1.0 / (1.0 + np.exp(-z))            # [K, J, N, T]
    wv = W.reshape(K * J, N * T).astype(np.float32)
    av = (1.0 - W).reshape(K * J, N * T).astype(np.float32)
    return wv, av


def kernel(x, w_start, w_end):
    wv, av = _host_prep(w_start, w_end)
    nc = _build_graph()
    x = np.ascontiguousarray(x, dtype=np.float32)
    in_maps = [
        {"x": x[i * B:(i + 1) * B], "wv": wv, "av": av}
        for i in range(N_CORES)
    ]
    try:
        res = run_bass_kernel_spmd(nc, in_maps, list(range(N_CORES))).results
    except Exception:
        # transient device/RPC failures have been observed; retry once
        res = run_bass_kernel_spmd(nc, in_maps, list(range(N_CORES))).results
    return np.concatenate([res[i]["out"] for i in range(N_CORES)], axis=0)
